# revision 7
# baseline (speedup 1.0000x reference)
"""Trainium2 Bass kernel for DeformConvTranspose1d.

Problem (hardcoded): B=8, Cin=256, Win=4096, Cout=256, K=4, stride=2, pad=1,
out_pad=0, dil=1, groups=1, offset_groups=1 -> Wout=8192.

Math:
  cols[b,co,k,i] = sum_ci x[b,ci,i] * weight[ci,co,k]
  pos = i*2 - 1 + k + offset[b,k,i]
  out[b,co,j] = bias[co] + sum_{k,i} cols[b,co,k,i] * mask[b,k,i] * hat(j - pos)
  where hat(u) = max(0, 1 - |u|)   (linear-interp scatter == hat kernel)

Strategy: data-parallel over batch, 1 sample per NeuronCore (8 cores).
Per core, loop over 32 chunks of 128 input positions:
  - GEMM1 (TensorE, fp32r): cols_T[i, (k,co)] = x_chunk^T @ W     [128 x 1024]
  - modulate by mask (ScalarE copy-with-scale + VectorE tensor_scalar)
  - build scatter matrix S[i, jl] = relu(1 - |iota_jl - pos_i|) over a local
    window of WLOC=304 output columns (VectorE |iota-pos| + ScalarE relu)
  - scatter matmul (TensorE, fp32r): pout[co, jl] += val^T[:,k] @ S_k
  - accumulate window into persistent out_sb with bias folded into the
    first (fresh) write of each column region.
Assumes |offset| < R=16 (offsets are N(0,1); max over this input ~4.9).
"""

import sys


import numpy as np

P = 128
B = 8
CIN = 256
WIN = 4096
CO = 256
K = 4
R = 16
WLOC = 304
OVL = WLOC - 256  # 48
N_CORES = 8

_nc_cache = {}


def build_nc(win=WIN, n_cores=N_CORES):
    import concourse.tile as tile
    from concourse import bacc, mybir

    f32 = mybir.dt.float32
    bf16 = mybir.dt.bfloat16
    Alu = mybir.AluOpType
    Act = mybir.ActivationFunctionType

    nch = win // P
    wout = (win - 1) * 2 - 2 + 3 + 1

    nc = bacc.Bacc("TRN2", target_bir_lowering=False, debug=False,
                   num_devices=n_cores)
    x_d = nc.dram_tensor("x", [2, P, win], bf16, kind="ExternalInput")
    w_d = nc.dram_tensor("wr", [2, P, K * CO], bf16, kind="ExternalInput")
    offT_d = nc.dram_tensor("offT", [win, K], f32, kind="ExternalInput")
    mT_d = nc.dram_tensor("mT", [win, K], f32, kind="ExternalInput")
    bias_d = nc.dram_tensor("biasr", [2, P], f32, kind="ExternalInput")
    out_d = nc.dram_tensor("out", [CO, wout], f32, kind="ExternalOutput")

    with tile.TileContext(nc) as tc:
        with (
            tc.tile_pool(name="const", bufs=1) as constp,
            tc.tile_pool(name="outp", bufs=1) as outp,
            tc.tile_pool(name="xin", bufs=3) as xin,
            tc.tile_pool(name="val", bufs=3) as valp,
            tc.tile_pool(name="tbuf", bufs=3) as tbp,
            tc.tile_pool(name="sbuf_s", bufs=3) as sp,
            tc.tile_pool(name="pcols", bufs=2, space="PSUM") as pcols,
            tc.tile_pool(name="pout", bufs=2, space="PSUM") as poutp,
        ):
            w_sb = constp.tile([P, 2, K * CO], bf16)
            for h in range(2):
                nc.sync.dma_start(out=w_sb[:, h, :], in_=w_d.ap()[h])
            offT_sb = constp.tile([P, nch, K], f32)
            nc.sync.dma_start(
                out=offT_sb[:],
                in_=offT_d.ap().rearrange("(c p) k -> p c k", p=P))
            mT_sb = constp.tile([P, nch, K], f32)
            nc.sync.dma_start(
                out=mT_sb[:],
                in_=mT_d.ap().rearrange("(c p) k -> p c k", p=P))
            bias_sb = constp.tile([P, 2], f32)
            nc.sync.dma_start(out=bias_sb[:],
                              in_=bias_d.ap().rearrange("h p -> p h"))
            iota_f = constp.tile([P, WLOC], f32)
            nc.gpsimd.iota(iota_f[:], pattern=[[1, WLOC]], base=0,
                           channel_multiplier=0,
                           allow_small_or_imprecise_dtypes=True)
            negposb = constp.tile([P, nch, K], f32)
            nc.gpsimd.iota(negposb[:], pattern=[[0, nch], [-1, K]], base=-R,
                           channel_multiplier=-2,
                           allow_small_or_imprecise_dtypes=True)
            negpos_all = constp.tile([P, nch, K], f32)
            nc.vector.tensor_tensor(out=negpos_all[:], in0=negposb[:],
                                    in1=offT_sb[:], op=Alu.subtract)
            out_sb = outp.tile([P, 2, wout], f32)

            dma_done = 0
            for c in range(nch):
                x_t = xin.tile([P, 2, P], bf16)
                for h in range(2):
                    nc.sync.dma_start(out=x_t[:, h, :],
                                      in_=x_d.ap()[h, :, c * P:(c + 1) * P])
                cols_ps = pcols.tile([P, K * CO], f32)
                for h in range(2):
                    for n in range(2):
                        nc.tensor.matmul(
                            out=cols_ps[:, n * 512:(n + 1) * 512],
                            lhsT=x_t[:, h, :],
                            rhs=w_sb[:, h, n * 512:(n + 1) * 512],
                            start=(h == 0), stop=(h == 1))
                val_sb = valp.tile([P, K * CO], bf16)
                for k in range(K):
                    sl = slice(k * CO, (k + 1) * CO)
                    m_col = mT_sb[:, c, k:k + 1]
                    if k < 1:
                        nc.scalar.activation(out=val_sb[:, sl],
                                             in_=cols_ps[:, sl],
                                             func=Act.Copy, scale=m_col)
                    else:
                        nc.vector.tensor_scalar(out=val_sb[:, sl],
                                                in0=cols_ps[:, sl],
                                                scalar1=m_col, scalar2=None,
                                                op0=Alu.mult)
                t_all = tbp.tile([P, K, WLOC], f32)
                for k in range(K):
                    nc.scalar.activation(out=t_all[:, k, :], in_=iota_f[:],
                                         func=Act.Abs,
                                         bias=negpos_all[:, c, k:k + 1],
                                         scale=1.0)
                # s = min(t,1) - 1 = -hat  (negated; sign fixed in accumulate)
                s_all = sp.tile([P, K, WLOC], bf16)
                for k in range(K):
                    nc.vector.tensor_scalar(out=s_all[:, k, :],
                                            in0=t_all[:, k, :],
                                            scalar1=1.0, scalar2=1.0,
                                            op0=Alu.min, op1=Alu.subtract)
                pouts = []
                for ch in range(2):
                    po = poutp.tile([P, WLOC], f32, tag=f"pout{ch}")
                    for k in range(K):
                        lo = k * CO + ch * P
                        nc.tensor.matmul(out=po[:],
                                         lhsT=val_sb[:, lo:lo + P],
                                         rhs=s_all[:, k, :],
                                         start=(k == 0), stop=(k == K - 1))
                    pouts.append(po)
                jbase = 256 * c - 1 - R
                for ch in range(2):
                    po = pouts[ch]
                    bcol = bias_sb[:, ch:ch + 1]
                    if c == 0:
                        fsl = (slice(0, WLOC - 1 - R), slice(1 + R, WLOC))
                    else:
                        # overlap: out_sb -= po  (po is negated contribution)
                        nc.vector.tensor_tensor(
                            out=out_sb[:, ch, jbase:jbase + OVL],
                            in0=out_sb[:, ch, jbase:jbase + OVL],
                            in1=po[:, 0:OVL], op=Alu.subtract)
                        fe = min(jbase + WLOC, wout)
                        fsl = (slice(jbase + OVL, fe),
                               slice(OVL, OVL + (fe - (jbase + OVL))))
                    # fresh: out_sb = bias - po
                    if ch == 0:
                        nc.scalar.activation(
                            out=out_sb[:, ch, fsl[0]], in_=po[:, fsl[1]],
                            func=Act.Identity, bias=bcol, scale=-1.0)
                    else:
                        nc.vector.tensor_scalar(
                            out=out_sb[:, ch, fsl[0]], in0=po[:, fsl[1]],
                            scalar1=bcol, scalar2=-1.0,
                            op0=Alu.subtract, op1=Alu.mult)
                if c % 4 == 3 or c == nch - 1:
                    end = wout if c == nch - 1 else 256 * (c + 1) - 1 - R
                    if end > dma_done:
                        for ch in range(2):
                            nc.sync.dma_start(
                                out=out_d.ap()[ch * P:(ch + 1) * P,
                                               dma_done:end],
                                in_=out_sb[:, ch, dma_done:end])
                        dma_done = end
    nc.compile()
    return nc


def _get_nc():
    key = (WIN, N_CORES)
    if key not in _nc_cache:
        _nc_cache[key] = build_nc(WIN, N_CORES)
    return _nc_cache[key]


def make_in_maps(x, weight, offset, mask, bias, win=WIN):
    import ml_dtypes
    bf = ml_dtypes.bfloat16
    wr = np.ascontiguousarray(
        np.transpose(weight, (0, 2, 1)).reshape(2, P, K * CO)).astype(bf)
    biasr = np.ascontiguousarray(bias.reshape(2, P))
    in_maps = []
    for b in range(x.shape[0]):
        in_maps.append({
            "x": np.ascontiguousarray(x[b].reshape(2, P, win)).astype(bf),
            "wr": wr,
            "offT": np.ascontiguousarray(offset[b].T),
            "mT": np.ascontiguousarray(mask[b].T),
            "biasr": biasr,
        })
    return in_maps


TRACE = False
last_results = None


def kernel(x, weight, offset, mask, bias):
    global last_results
    from concourse.bass_utils import run_bass_kernel_spmd

    x = np.asarray(x, dtype=np.float32)
    weight = np.asarray(weight, dtype=np.float32)
    offset = np.asarray(offset, dtype=np.float32)
    mask = np.asarray(mask, dtype=np.float32)
    bias = np.asarray(bias, dtype=np.float32)

    nc = _get_nc()
    in_maps = make_in_maps(x, weight, offset, mask, bias)
    res = run_bass_kernel_spmd(nc, in_maps, core_ids=list(range(N_CORES)),
                               trace=TRACE)
    last_results = res
    return np.stack([res.results[b]["out"] for b in range(B)])


# revision 19
# speedup vs baseline: 1.2211x; 1.2211x over previous
"""Trainium2 Bass kernel for DeformConvTranspose1d.

Problem (hardcoded): B=8, Cin=256, Win=4096, Cout=256, K=4, stride=2, pad=1,
out_pad=0, dil=1, groups=1, offset_groups=1 -> Wout=8192.

Math:
  cols[b,co,k,i] = sum_ci x[b,ci,i] * weight[ci,co,k]
  pos = i*2 - 1 + k + offset[b,k,i]
  out[b,co,j] = bias[co] + sum_{k,i} cols[b,co,k,i] * mask[b,k,i] * hat(j - pos)
  where hat(u) = max(0, 1 - |u|)   (linear-interp scatter == hat kernel)

Strategy: data-parallel over batch, 1 sample per NeuronCore (8 cores).
Per core, loop over 32 chunks of 128 input positions (all matmul operands
bf16, PSUM accumulation fp32):
  - GEMM1 (TensorE): cols_T[i, (k,co)] = x_chunk^T @ W   [128 x 1024] in PSUM
  - copy cols PSUM->SBUF as bf16 val (split ScalarE/VectorE)
  - hat build over a WLOC=288-column local output window:
      ScalarE:  t' = Abs(m*iota - m*pos)        (mask m folded in)
      GPSIMD:   s  = min(t', m) - m  = -m*hat   (negated hat weights)
  - scatter matmul (TensorE): po[co, jl] -= sum_k val_k^T @ s_k
  - accumulate window into persistent out_sb (VectorE), subtracting po
    (fixes the negation) with bias folded into the first write of each
    column region; stream finished 1024-col blocks to DRAM.
Assumes |offset| < R=12 (offsets are N(0,1); max over this input ~4.9).
"""

import sys


import numpy as np

P = 128
B = 8
CIN = 256
WIN = 4096
CO = 256
K = 4
R = 12
WLOC = 288
OVL = WLOC - 256  # 48
N_CORES = 8

_nc_cache = {}


def build_nc(win=WIN, n_cores=N_CORES):
    import concourse.tile as tile
    from concourse import bacc, mybir

    f32 = mybir.dt.float32
    bf16 = mybir.dt.bfloat16
    Alu = mybir.AluOpType
    Act = mybir.ActivationFunctionType

    nch = win // P
    wout = (win - 1) * 2 - 2 + 3 + 1

    nc = bacc.Bacc("TRN2", target_bir_lowering=False, debug=False,
                   num_devices=n_cores)
    x_d = nc.dram_tensor("x", [2, P, win], bf16, kind="ExternalInput")
    w_d = nc.dram_tensor("wr", [2, P, K * CO], bf16, kind="ExternalInput")
    offT_d = nc.dram_tensor("offT", [win, K], f32, kind="ExternalInput")
    mT_d = nc.dram_tensor("mT", [win, K], f32, kind="ExternalInput")
    bias_d = nc.dram_tensor("biasr", [2, P], f32, kind="ExternalInput")
    out_d = nc.dram_tensor("out", [CO, wout], f32, kind="ExternalOutput")

    with tile.TileContext(nc) as tc:
        with (
            tc.tile_pool(name="const", bufs=1) as constp,
            tc.tile_pool(name="outp", bufs=1) as outp,
            tc.tile_pool(name="xin", bufs=4) as xin,
            tc.tile_pool(name="val", bufs=4) as valp,
            tc.tile_pool(name="tbuf", bufs=4) as tbp,
            tc.tile_pool(name="sbuf_s", bufs=4) as sp,
            tc.tile_pool(name="pcols", bufs=2, space="PSUM") as pcols,
            tc.tile_pool(name="pout", bufs=2, space="PSUM") as poutp,
        ):
            w_sb = constp.tile([P, 2, K * CO], bf16)
            for h in range(2):
                nc.sync.dma_start(out=w_sb[:, h, :], in_=w_d.ap()[h])
            offT_sb = constp.tile([P, nch, K], f32)
            nc.sync.dma_start(
                out=offT_sb[:],
                in_=offT_d.ap().rearrange("(c p) k -> p c k", p=P))
            mT_sb = constp.tile([P, nch, K], f32)
            nc.sync.dma_start(
                out=mT_sb[:],
                in_=mT_d.ap().rearrange("(c p) k -> p c k", p=P))
            bias_sb = constp.tile([P, 2], f32)
            nc.sync.dma_start(out=bias_sb[:],
                              in_=bias_d.ap().rearrange("h p -> p h"))
            iota_f = constp.tile([P, WLOC], f32)
            nc.gpsimd.iota(iota_f[:], pattern=[[1, WLOC]], base=0,
                           channel_multiplier=0,
                           allow_small_or_imprecise_dtypes=True)
            negposb = constp.tile([P, nch, K], f32)
            nc.gpsimd.iota(negposb[:], pattern=[[0, nch], [-1, K]], base=-R,
                           channel_multiplier=-2,
                           allow_small_or_imprecise_dtypes=True)
            negpos_all = constp.tile([P, nch, K], f32)
            nc.vector.tensor_tensor(out=negpos_all[:], in0=negposb[:],
                                    in1=offT_sb[:], op=Alu.subtract)
            negmpos_all = constp.tile([P, nch, K], f32)
            nc.vector.tensor_tensor(out=negmpos_all[:], in0=negpos_all[:],
                                    in1=mT_sb[:], op=Alu.mult)
            out_sb = outp.tile([P, 2, wout], f32)

            dma_done = 0
            for c in range(nch):
                x_t = xin.tile([P, 2, P], bf16)
                for h in range(2):
                    nc.sync.dma_start(out=x_t[:, h, :],
                                      in_=x_d.ap()[h, :, c * P:(c + 1) * P])
                cols_ps = pcols.tile([P, K * CO], f32)
                for h in range(2):
                    for n in range(2):
                        nc.tensor.matmul(
                            out=cols_ps[:, n * 512:(n + 1) * 512],
                            lhsT=x_t[:, h, :],
                            rhs=w_sb[:, h, n * 512:(n + 1) * 512],
                            start=(h == 0), stop=(h == 1))
                val_sb = valp.tile([P, K * CO], bf16)
                nc.scalar.activation(out=val_sb[:, 0:256],
                                     in_=cols_ps[:, 0:256],
                                     func=Act.Copy, scale=1.0)
                nc.vector.tensor_scalar(out=val_sb[:, 256:1024],
                                        in0=cols_ps[:, 256:1024],
                                        scalar1=1.0, scalar2=None,
                                        op0=Alu.mult)
                # t' = |m*jl - m*pos|  (mask folded into the hat build)
                t_all = tbp.tile([P, K, WLOC], f32)
                for k in range(K):
                    nc.scalar.activation(out=t_all[:, k, :], in_=iota_f[:],
                                         func=Act.Abs,
                                         bias=negmpos_all[:, c, k:k + 1],
                                         scale=mT_sb[:, c, k:k + 1])
                # s = min(t',m) - m = -m*hat  (negated; sign fixed on accumulate)
                s_all = sp.tile([P, K, WLOC], bf16)
                for k in range(K):
                    m_col = mT_sb[:, c, k:k + 1]
                    nc.gpsimd.tensor_scalar(out=s_all[:, k, :],
                                            in0=t_all[:, k, :],
                                            scalar1=m_col, scalar2=m_col,
                                            op0=Alu.min, op1=Alu.subtract)
                po = poutp.tile([P, 2, 512], f32)
                for ch in range(2):
                    for k in range(K):
                        lo = k * CO + ch * P
                        nc.tensor.matmul(out=po[:, ch, 0:WLOC],
                                         lhsT=val_sb[:, lo:lo + P],
                                         rhs=s_all[:, k, :],
                                         start=(k == 0), stop=(k == K - 1))
                jbase = 256 * c - 1 - R
                bcol = bias_sb[:, 0:1]
                if c == 0:
                    fsl = (slice(0, WLOC - 1 - R), slice(1 + R, WLOC))
                else:
                    # overlap: out_sb -= po  (po is negated contribution)
                    nc.vector.tensor_tensor(
                        out=out_sb[:, :, jbase:jbase + OVL],
                        in0=out_sb[:, :, jbase:jbase + OVL],
                        in1=po[:, :, 0:OVL], op=Alu.subtract)
                    fe = min(jbase + WLOC, wout)
                    fsl = (slice(jbase + OVL, fe),
                           slice(OVL, OVL + (fe - (jbase + OVL))))
                # fresh: out_sb = bias - po  (both ch halves in one op)
                wfr = fsl[0].stop - fsl[0].start
                nc.vector.tensor_tensor(
                    out=out_sb[:, :, fsl[0]],
                    in0=bias_sb[:, 0:2].rearrange("p (a u) -> p a u", u=1)
                        .to_broadcast([P, 2, wfr]),
                    in1=po[:, :, fsl[1]], op=Alu.subtract)
                if c % 4 == 3 or c == nch - 1:
                    end = wout if c == nch - 1 else 256 * (c + 1) - 1 - R
                    if end > dma_done:
                        for ch in range(2):
                            nc.sync.dma_start(
                                out=out_d.ap()[ch * P:(ch + 1) * P,
                                               dma_done:end],
                                in_=out_sb[:, ch, dma_done:end])
                        dma_done = end
    nc.compile()
    return nc


def _get_nc():
    key = (WIN, N_CORES)
    if key not in _nc_cache:
        _nc_cache[key] = build_nc(WIN, N_CORES)
    return _nc_cache[key]


def make_in_maps(x, weight, offset, mask, bias, win=WIN):
    import ml_dtypes
    bf = ml_dtypes.bfloat16
    wr = np.ascontiguousarray(
        np.transpose(weight, (0, 2, 1)).reshape(2, P, K * CO)).astype(bf)
    biasr = np.ascontiguousarray(bias.reshape(2, P))
    in_maps = []
    for b in range(x.shape[0]):
        in_maps.append({
            "x": np.ascontiguousarray(x[b].reshape(2, P, win)).astype(bf),
            "wr": wr,
            "offT": np.ascontiguousarray(offset[b].T),
            "mT": np.ascontiguousarray(mask[b].T),
            "biasr": biasr,
        })
    return in_maps


TRACE = False
last_results = None


def kernel(x, weight, offset, mask, bias):
    global last_results
    from concourse.bass_utils import run_bass_kernel_spmd

    x = np.asarray(x, dtype=np.float32)
    weight = np.asarray(weight, dtype=np.float32)
    offset = np.asarray(offset, dtype=np.float32)
    mask = np.asarray(mask, dtype=np.float32)
    bias = np.asarray(bias, dtype=np.float32)

    nc = _get_nc()
    in_maps = make_in_maps(x, weight, offset, mask, bias)
    res = run_bass_kernel_spmd(nc, in_maps, core_ids=list(range(N_CORES)),
                               trace=TRACE)
    last_results = res
    return np.stack([res.results[b]["out"] for b in range(B)])
